# revision 30
# baseline (speedup 1.0000x reference)
"""Trainium2 Bass kernel for nn_CausalLinearAttention (B=4, T=4096, D=1024, H=16).

Sharding: 8 cores = 4 batches x 2 head-shards (8 heads each). Each core:
  - computes Q/V projections for its 512 local features, K projection for all
    1024 features (needed by the head-axis cumsum in the normalizer Z),
  - builds the head-cumsum state R in PSUM via selector matmuls (the cross-shard
    prefix base is folded in by a flag baked into the per-core selector data),
  - folds Z into Q (linearity), runs the chunked causal scan with per-head
    64x64 state, and emits a partial output projection.
Host sums the two head-shard partials per batch and adds the bias.

Self-contained: hardcodes shapes; only needs the concourse repo on sys.path.
"""
import sys

for _p in ("/opt/trn_rl_repo", "/root/.axon_site/_ro/trn_rl_repo"):
    if _p not in sys.path:
        sys.path.append(_p)

import numpy as np
import ml_dtypes

import concourse.bass as bass
from concourse import bacc, mybir
from concourse.bass_utils import run_bass_kernel_spmd
from concourse.tile import TileContext

bf16 = mybir.dt.bfloat16
f32 = mybir.dt.float32
AF = mybir.ActivationFunctionType
ALU = mybir.AluOpType

B, T, D = 4, 4096, 1024
H, DH = 16, 64
HL = 8            # local heads
FL = HL * DH      # 512 local features
CHUNK = 128
EPS = 1e-6
NBLK = 4
TB = T // NBLK    # 1024 tokens per block
NT = TB // CHUNK  # 8 token tiles per block
KD = D // 128     # 8 contraction tiles over d_model


def _feature(nc, pool, psum, out_ap, n):
    """out = relu(psum) + min(exp(psum), 1) over [128, n]."""
    e = pool.tile([128, 512], f32, tag="feat_e")
    nc.scalar.activation(e[:, 0:n], psum, AF.Exp)
    nc.vector.tensor_scalar_min(e[:, 0:n], e[:, 0:n], 1.0)
    nc.vector.scalar_tensor_tensor(out=out_ap, in0=psum, scalar=0.0, in1=e[:, 0:n],
                                   op0=ALU.max, op1=ALU.add)


def build_nc(t_total=T, nblk=NBLK, phases=("q", "ko", "kl", "z", "tr", "v", "scan", "proj"), debug=False, repeat=1, use_cc=False):
    tb = t_total // nblk
    nt = tb // CHUNK
    nc = bacc.Bacc("TRN2", target_bir_lowering=False, debug=debug)

    xt = nc.dram_tensor("xt", [128, KD, t_total], bf16, kind="ExternalInput")
    wq = nc.dram_tensor("wq", [128, KD, FL], bf16, kind="ExternalInput")
    wk = nc.dram_tensor("wk", [128, KD, D], bf16, kind="ExternalInput")
    wv = nc.dram_tensor("wv", [128, KD, FL], bf16, kind="ExternalInput")
    wo = nc.dram_tensor("wo", [128, 4, D], bf16, kind="ExternalInput")
    mask_d = nc.dram_tensor("mask", [128, 128], f32, kind="ExternalInput")
    mask2_d = nc.dram_tensor("mask2", [128, 256], f32, kind="ExternalInput")
    ident_d = nc.dram_tensor("ident", [128, 128], bf16, kind="ExternalInput")
    sel_other_d = nc.dram_tensor("sel_other", [128, 128], bf16, kind="ExternalInput")
    sel_sum_d = nc.dram_tensor("sel_sum", [128, 128], bf16, kind="ExternalInput")
    sel_f_d = nc.dram_tensor("sel_f", [128, 128], bf16, kind="ExternalInput")
    sel_lo_d = nc.dram_tensor("sel_lo", [128, 128], bf16, kind="ExternalInput")
    sel_hi_d = nc.dram_tensor("sel_hi", [128, 128], bf16, kind="ExternalInput")
    zsel_d = nc.dram_tensor("zsel", [128, 128], bf16, kind="ExternalInput")
    y = nc.dram_tensor("y", [t_total, D], f32, kind="ExternalOutput")

    from contextlib import ExitStack
    with TileContext(nc) as tc:
        with ExitStack() as stack:
            pools = {}
            for pname, pbufs, pspace in (
                ("consts", 1, "SBUF"), ("xtp", 2, "SBUF"), ("qtfp", 1, "SBUF"),
                ("qtbp", 2, "SBUF"), ("ktp", 2, "SBUF"), ("kop", 2, "SBUF"),
                ("knp", 1, "SBUF"), ("vnp", 2, "SBUF"), ("otp", 1, "SBUF"),
                ("tmpp", 2, "SBUF"), ("featp", 2, "SBUF"), ("zp", 2, "SBUF"),
                ("attnp", 3, "SBUF"), ("sp", 1, "SBUF"), ("ystp", 2, "SBUF"),
                ("ccp", 2, "SBUF"), ("dramp", 2, "DRAM"),
                ("ps_big", 2, "PSUM"), ("ps_r", 2, "PSUM"),
                ("ps_attn", 2, "PSUM"), ("ps_sm", 2, "PSUM"),
            ):
                pools[pname] = stack.enter_context(
                    tc.tile_pool(name=pname, bufs=pbufs, space=pspace))
            consts, xtp, qtfp, qtbp, ktp, kop, knp, vnp, otp, tmpp, featp, zp, \
                attnp, sp, ystp, ccp, dramp, ps_big, ps_r, ps_attn, ps_sm = (
                    pools[n] for n in (
                        "consts", "xtp", "qtfp", "qtbp", "ktp", "kop", "knp",
                        "vnp", "otp", "tmpp", "featp", "zp", "attnp", "sp",
                        "ystp", "ccp", "dramp", "ps_big", "ps_r", "ps_attn", "ps_sm"))
            # ---- constants / weights
            wq_s = consts.tile([128, KD, FL], bf16, tag="wq_s")
            wk_s = consts.tile([128, KD, D], bf16, tag="wk_s")
            wv_s = consts.tile([128, KD, FL], bf16, tag="wv_s")
            wo_s = consts.tile([128, 4, D], bf16, tag="wo_s")
            mask_s = consts.tile([128, 128], f32, tag="mask_s")
            mask2_s = consts.tile([128, 256], f32, tag="mask2_s")
            ident_s = consts.tile([128, 128], bf16, tag="ident_s")
            sel_other_s = consts.tile([128, 128], bf16, tag="sel_other_s")
            sel_sum_s = consts.tile([128, 128], bf16, tag="sel_sum_s")
            sel_f_s = consts.tile([128, 128], bf16, tag="sel_f_s")
            sel_lo_s = consts.tile([128, 128], bf16, tag="sel_lo_s")
            sel_hi_s = consts.tile([128, 128], bf16, tag="sel_hi_s")
            zsel_s = consts.tile([128, 128], bf16, tag="zsel_s")
            xtb0 = xtp.tile([128, KD, tb], bf16, tag="xtb", name="xtb0")
            nc.sync.dma_start(out=xtb0[:, :, 0:512], in_=xt[:, :, 0:512])
            nc.sync.dma_start(out=xtb0[:, :, 512:tb], in_=xt[:, :, 512:tb])
            for dst, src in ((wk_s, wk), (wq_s, wq), (wv_s, wv), (wo_s, wo),
                             (mask_s, mask_d), (ident_s, ident_d),
                             (mask2_s, mask2_d),
                             (sel_other_s, sel_other_d), (sel_sum_s, sel_sum_d),
                             (sel_f_s, sel_f_d), (sel_lo_s, sel_lo_d),
                             (sel_hi_s, sel_hi_d), (zsel_s, zsel_d)):
                nc.gpsimd.dma_start(out=dst, in_=src[tuple([slice(None)] * len(src.shape))])

            # ---- persistent scan state
            s_f32 = sp.tile([128, 4, DH], f32, tag="s_f32")
            s_bf = sp.tile([128, 4, DH], bf16, tag="s_bf")

            for rep in range(repeat):
              for bi in range(nblk):
                tsl = slice(bi * tb, (bi + 1) * tb)
                if bi == 0 and rep == 0:
                    xtb = xtb0
                else:
                    xtb = xtp.tile([128, KD, tb], bf16, tag="xtb")
                    nc.sync.dma_start(out=xtb, in_=xt[:, :, tsl])

                # ---- K-local pass: kt, R prefix, z, qtb
                kt = ktp.tile([128, 4, tb], bf16, tag="kt")
                qtb = qtbp.tile([128, 4, tb], bf16, tag="qtb")
                r_ps = [ps_r.tile([128, 512], f32, tag="ps_r", name=f"r_ps{j}")
                        for j in range(2)]
                if "kl" not in phases:
                    nc.vector.memset(kt, 0.01)
                nck = tb // 512
                for m in range(4 if "kl" in phases else 0):
                    for nch in range(nck):
                        csl = slice(nch * 512, (nch + 1) * 512)
                        pk = ps_big.tile([128, 512], f32, tag="ps_big")
                        for k in range(KD):
                            nc.tensor.matmul(pk, wk_s[:, k, (4 + m) * 128:(5 + m) * 128],
                                             xtb[:, k, csl], start=(k == 0), stop=(k == KD - 1))
                        _feature(nc, featp, pk, kt[:, m, csl], 512)
                if use_cc and "kl" in phases and "z" in phases:
                    s_own = ccp.tile([64, tb], bf16, tag="s_own")
                    for nch in range(nck):
                        csl = slice(nch * 512, (nch + 1) * 512)
                        sown = ps_big.tile([64, 512], f32, tag="ps_big", name="sown")
                        for m in range(4):
                            nc.tensor.matmul(sown, sel_sum_s[:, 0:64], kt[:, m, csl],
                                             start=(m == 0), stop=(m == 3))
                        nc.scalar.copy(s_own[:, csl], sown)
                    cc_in = dramp.tile([64, tb], bf16, tag="cc_in")
                    cc_out = dramp.tile([64, tb], bf16, tag="cc_out")
                    nc.sync.dma_start(out=cc_in[:, :], in_=s_own)
                    nc.gpsimd.collective_compute(
                        "AllReduce", ALU.add,
                        replica_groups=[[0, 1], [2, 3], [4, 5], [6, 7]],
                        ins=[cc_in[:, :]], outs=[cc_out[:, :]])
                    s_tot = ccp.tile([64, tb], bf16, tag="s_tot")
                    nc.sync.dma_start(out=s_tot, in_=cc_out[:, :])
                    s_oth = ccp.tile([64, tb], bf16, tag="s_oth")
                    nc.vector.tensor_sub(s_oth, s_tot, s_own)
                    for nch in range(nck):
                        csl = slice(nch * 512, (nch + 1) * 512)
                        nc.tensor.matmul(r_ps[nch], sel_f_s[0:64, :], s_oth[:, csl],
                                         start=True, stop=False, skip_group_check=True)
                # ---- Q projection + feature -> qt_f32 [128, 4, TB]
                qt_f32 = qtfp.tile([128, 4, tb], f32, tag="qt_f32")
                if "q" not in phases:
                    nc.vector.memset(qt_f32, 0.01)
                for m in range(4 if "q" in phases else 0):
                    for nch in range(tb // 512):
                        csl = slice(nch * 512, (nch + 1) * 512)
                        pq = ps_big.tile([128, 512], f32, tag="ps_big")
                        for k in range(KD):
                            nc.tensor.matmul(pq, wq_s[:, k, m * 128:(m + 1) * 128],
                                             xtb[:, k, csl], start=(k == 0), stop=(k == KD - 1))
                        _feature(nc, featp, pq, qt_f32[:, m, csl], 512)

                # ---- K-other pass -> R accumulation (R on both halves, in PSUM)
                if "z" not in phases:
                    nc.vector.tensor_copy(qtb, qt_f32)
                for m in range(4 if ("ko" in phases and not use_cc) else 0):
                    ko = kop.tile([128, tb], bf16, tag="ko")
                    for nch in range(tb // 512):
                        csl = slice(nch * 512, (nch + 1) * 512)
                        pk = ps_big.tile([128, 512], f32, tag="ps_big")
                        for k in range(KD):
                            nc.tensor.matmul(pk, wk_s[:, k, m * 128:(m + 1) * 128],
                                             xtb[:, k, csl], start=(k == 0), stop=(k == KD - 1))
                        _feature(nc, featp, pk, ko[:, csl], 512)
                    for nch in range(tb // 512 if "z" in phases else 0):
                        csl = slice(nch * 512, (nch + 1) * 512)
                        nc.tensor.matmul(r_ps[nch], sel_other_s, ko[:, csl],
                                         start=(m == 0), stop=False, skip_group_check=True)

                # ---- knat via PE transposes: [128(tok), NT, 512(feat)]
                knat = knp.tile([128, nt, FL], bf16, tag="knat")
                if "tr" not in phases:
                    nc.vector.memset(knat, 0.01)
                for tt in range(nt if "tr" in phases else 0):
                    ptr = ps_attn.tile([128, 512], bf16, tag="ps_attn")
                    for m in range(4):
                        nc.tensor.transpose(ptr[:, m * 128:(m + 1) * 128],
                                            kt[:, m, tt * 128:(tt + 1) * 128], ident_s)
                    nc.vector.tensor_copy(knat[:, tt, :], ptr)

                # ---- V pass -> vnat [128(tok), NT, 512]
                vnat = vnp.tile([128, nt, FL], bf16, tag="vnat")
                if "v" not in phases:
                    nc.vector.memset(vnat, 0.01)
                for tt in range(nt if "v" in phases else 0):
                    pv = ps_big.tile([128, 512], f32, tag="ps_big")
                    for k in range(KD):
                        nc.tensor.matmul(pv, xtb[:, k, tt * 128:(tt + 1) * 128],
                                         wv_s[:, k, :], start=(k == 0), stop=(k == KD - 1))
                    nc.scalar.copy(vnat[:, tt, :], pv)

                for m in range(4 if ("kl" in phases and "z" in phases) else 0):
                    if False:
                        pass
                    tmp = tmpp.tile([128, tb], bf16, tag="tmp")
                    for nch in range(tb // 512):
                        csl = slice(nch * 512, (nch + 1) * 512)
                        nc.tensor.matmul(r_ps[nch], sel_lo_s, kt[:, m, csl],
                                         start=(m == 0 and not use_cc and "ko" not in phases),
                                         stop=False, skip_group_check=True)
                        nc.vector.tensor_mul(tmp[0:64, csl], qt_f32[0:64, m, csl], r_ps[nch][0:64, :])
                    for nch in range(tb // 512):
                        csl = slice(nch * 512, (nch + 1) * 512)
                        nc.tensor.matmul(r_ps[nch], sel_hi_s, kt[:, m, csl],
                                         start=False, stop=(m == 3), skip_group_check=True)
                        nc.vector.tensor_mul(tmp[64:128, csl], qt_f32[64:128, m, csl], r_ps[nch][64:128, :])
                    for nch in range(tb // 512):
                        csl = slice(nch * 512, (nch + 1) * 512)
                        pz = ps_big.tile([128, 512], f32, tag="ps_big")
                        nc.tensor.matmul(pz, zsel_s, tmp[:, csl], start=True, stop=True)
                        zt = zp.tile([128, 512], f32, tag="zt")
                        nc.vector.reciprocal(zt, pz)
                        nc.gpsimd.tensor_mul(qtb[:, m, csl], qt_f32[:, m, csl], zt)

                # ---- causal scan
                outT = otp.tile([128, 4, tb], bf16, tag="outT")
                if "scan" not in phases:
                    nc.vector.memset(outT, 0.01)
                for tt in range(nt if "scan" in phases else 0):
                    c = bi * nt + tt
                    if c > 0:
                        nc.gpsimd.tensor_copy(s_bf, s_f32)
                    csl = slice(tt * 128, (tt + 1) * 128)
                    for pl in range(4):
                        pp = ps_sm.tile([128, 128], f32, tag="ps_sm")
                        attn_pair = []
                        for h in range(2):
                            i = 2 * pl + h
                            b64 = 64 * h
                            pa = ps_attn.tile([128, 128], f32, tag="ps_attn", name=f"pa{h}")
                            nc.tensor.matmul(pa, kt[b64:b64 + 64, pl, csl],
                                             qtb[b64:b64 + 64, pl, csl], start=True, stop=True)
                            attn_s = attnp.tile([128, 128], bf16, tag="attn_s", name=f"attn_s{h}")
                            nc.vector.tensor_mul(attn_s, pa, mask_s)
                            attn_pair.append(attn_s)
                        for h in range(2):
                            i = 2 * pl + h
                            b64 = 64 * h
                            nc.tensor.matmul(pp[b64:b64 + 64, :],
                                             vnat[:, tt, i * 64:(i + 1) * 64], attn_pair[h],
                                             start=True, stop=(c == 0), tile_position=(0, b64))
                            if c > 0:
                                nc.tensor.matmul(pp[b64:b64 + 64, :], s_bf[b64:b64 + 64, pl, :],
                                                 qtb[b64:b64 + 64, pl, csl],
                                                 start=False, stop=True, tile_position=(b64, b64))
                        nc.scalar.copy(outT[:, pl, csl], pp)
                        pd = ps_sm.tile([128, 64], f32, tag="ps_sm")
                        for h in range(2):
                            i = 2 * pl + h
                            b64 = 64 * h
                            nc.tensor.matmul(pd[b64:b64 + 64, :], knat[:, tt, i * 64:(i + 1) * 64],
                                             vnat[:, tt, i * 64:(i + 1) * 64],
                                             start=True, stop=True, tile_position=(0, b64))
                        if c == 0:
                            nc.vector.tensor_copy(s_f32[:, pl, :], pd)
                        else:
                            nc.vector.tensor_add(s_f32[:, pl, :], s_f32[:, pl, :], pd)

                # ---- output projection (partial over local features)
                for tt in range(nt if "proj" in phases else 0):
                    yst = ystp.tile([128, D], f32, tag="yst")
                    for nch in range(D // 512):
                        csl = slice(nch * 512, (nch + 1) * 512)
                        py = ps_big.tile([128, 512], f32, tag="ps_big")
                        for kf in range(4):
                            nc.tensor.matmul(py, outT[:, kf, tt * 128:(tt + 1) * 128],
                                             wo_s[:, kf, csl], start=(kf == 0), stop=(kf == 3))
                        nc.scalar.copy(yst[:, csl], py)
                    nc.sync.dma_start(out=y[bi * tb + tt * 128: bi * tb + (tt + 1) * 128, :],
                                      in_=yst)
    nc.finalize()
    return nc


_NC_CACHE = None


def _get_nc():
    global _NC_CACHE
    if _NC_CACHE is None:
        _NC_CACHE = build_nc()
    return _NC_CACHE


def _pack(w, kt):
    """[kt*128, N] -> [128, kt, N] contiguous."""
    return np.ascontiguousarray(w.reshape(kt, 128, -1).transpose(1, 0, 2))


def _core_inputs(x, Wq, Wk, Wv, Wo, core):
    b, s = core // 2, core % 2
    loc = np.arange(8 * s * DH, (8 * s + 8) * DH)
    oth = np.arange(8 * (1 - s) * DH, (8 * (1 - s) + 8) * DH)
    c16 = lambda a: np.ascontiguousarray(a.astype(ml_dtypes.bfloat16))

    I = np.eye(64, dtype=np.float32)
    Z = np.zeros((64, 64), dtype=np.float32)
    O = np.ones((64, 64), dtype=np.float32)
    flag = float(s)
    return {
        "xt": c16(_pack(x[b].T, KD)),
        "wq": c16(_pack(Wq[:, loc], KD)),
        "wk": c16(_pack(Wk[:, np.concatenate([oth, loc])], KD)),
        "wv": c16(_pack(Wv[:, loc], KD)),
        "wo": c16(_pack(Wo[loc, :], 4)),
        "mask": np.ascontiguousarray(np.triu(np.ones((128, 128), dtype=np.float32))),
        "mask2": np.ascontiguousarray(np.tile(np.triu(np.ones((128, 128), dtype=np.float32)), (1, 2))),
        "ident": c16(np.eye(128, dtype=np.float32)),
        "sel_other": c16(flag * np.block([[I, I], [I, I]])),
        "sel_sum": c16(np.block([[I, Z], [I, Z]])),
        "sel_f": c16(flag * np.block([[I, I], [Z, Z]])),
        "sel_lo": c16(np.block([[I, I], [Z, Z]])),
        "sel_hi": c16(np.block([[Z, Z], [I, I]])),
        "zsel": c16(np.block([[O, Z], [Z, O]])),
    }


def kernel(x, Wq, Wk, Wv, Wo, bo):
    x = np.asarray(x, dtype=np.float32)
    Wq = np.asarray(Wq, dtype=np.float32)
    Wk = np.asarray(Wk, dtype=np.float32)
    Wv = np.asarray(Wv, dtype=np.float32)
    Wo = np.asarray(Wo, dtype=np.float32)
    bo = np.asarray(bo, dtype=np.float32)

    nc = _get_nc()
    in_maps = [_core_inputs(x, Wq, Wk, Wv, Wo, core) for core in range(8)]
    res = run_bass_kernel_spmd(nc, in_maps, core_ids=list(range(8)))
    out = np.zeros((B, T, D), dtype=np.float32)
    for b in range(B):
        out[b] = (res.results[2 * b]["y"].astype(np.float32)
                  + res.results[2 * b + 1]["y"].astype(np.float32))
    out += bo
    return out


# revision 32
# speedup vs baseline: 8.0419x; 8.0419x over previous
"""Trainium2 Bass kernel for nn_CausalLinearAttention (B=4, T=4096, D=1024, H=16).

Sharding: 8 cores = 4 batches x 2 head-shards (8 heads each). Each core:
  - computes Q/V projections for its 512 local features, K projection for all
    1024 features (needed by the head-axis cumsum in the normalizer Z),
  - builds the head-cumsum state R in PSUM via selector matmuls (the cross-shard
    prefix base is folded in by a flag baked into the per-core selector data),
  - folds Z into Q (linearity), runs the chunked causal scan with per-head
    64x64 state, and emits a partial output projection.
Host sums the two head-shard partials per batch and adds the bias.

Self-contained: hardcodes shapes; only needs the concourse repo on sys.path.
"""
import sys

for _p in ("/opt/trn_rl_repo", "/root/.axon_site/_ro/trn_rl_repo"):
    if _p not in sys.path:
        sys.path.append(_p)

import numpy as np
import ml_dtypes

import concourse.bass as bass
from concourse import bacc, mybir
from concourse.bass_utils import run_bass_kernel_spmd
from concourse.tile import TileContext

bf16 = mybir.dt.bfloat16
f32 = mybir.dt.float32
AF = mybir.ActivationFunctionType
ALU = mybir.AluOpType

B, T, D = 4, 4096, 1024
H, DH = 16, 64
HL = 8            # local heads
FL = HL * DH      # 512 local features
CHUNK = 128
EPS = 1e-6
NBLK = 4
TB = T // NBLK    # 1024 tokens per block
NT = TB // CHUNK  # 8 token tiles per block
KD = D // 128     # 8 contraction tiles over d_model


def _feature(nc, pool, psum, out_ap, n):
    """out = relu(psum) + min(exp(psum), 1) over [128, n]."""
    e = pool.tile([128, 512], f32, tag="feat_e")
    nc.scalar.activation(e[:, 0:n], psum, AF.Exp)
    nc.vector.tensor_scalar_min(e[:, 0:n], e[:, 0:n], 1.0)
    nc.vector.scalar_tensor_tensor(out=out_ap, in0=psum, scalar=0.0, in1=e[:, 0:n],
                                   op0=ALU.max, op1=ALU.add)


def build_nc(t_total=T, nblk=NBLK, phases=("q", "ko", "kl", "z", "tr", "v", "scan", "proj"), debug=False, repeat=1, use_cc=False):
    tb = t_total // nblk
    nt = tb // CHUNK
    nc = bacc.Bacc("TRN2", target_bir_lowering=False, debug=debug)

    xt = nc.dram_tensor("xt", [128, KD, t_total], bf16, kind="ExternalInput")
    wq = nc.dram_tensor("wq", [128, KD, FL], bf16, kind="ExternalInput")
    wk = nc.dram_tensor("wk", [128, KD, D], bf16, kind="ExternalInput")
    wv = nc.dram_tensor("wv", [128, KD, FL], bf16, kind="ExternalInput")
    wo = nc.dram_tensor("wo", [128, 4, D], bf16, kind="ExternalInput")
    mask_d = nc.dram_tensor("mask", [128, 128], f32, kind="ExternalInput")
    mask2_d = nc.dram_tensor("mask2", [128, 256], f32, kind="ExternalInput")
    ident_d = nc.dram_tensor("ident", [128, 128], bf16, kind="ExternalInput")
    sel_other_d = nc.dram_tensor("sel_other", [128, 128], bf16, kind="ExternalInput")
    sel_sum_d = nc.dram_tensor("sel_sum", [128, 128], bf16, kind="ExternalInput")
    sel_f_d = nc.dram_tensor("sel_f", [128, 128], bf16, kind="ExternalInput")
    sel_lo_d = nc.dram_tensor("sel_lo", [128, 128], bf16, kind="ExternalInput")
    sel_hi_d = nc.dram_tensor("sel_hi", [128, 128], bf16, kind="ExternalInput")
    zsel_d = nc.dram_tensor("zsel", [128, 128], bf16, kind="ExternalInput")
    y = nc.dram_tensor("y", [t_total, D], f32, kind="ExternalOutput")

    from contextlib import ExitStack
    with TileContext(nc) as tc:
        with ExitStack() as stack:
            pools = {}
            for pname, pbufs, pspace in (
                ("consts", 1, "SBUF"), ("xtp", 2, "SBUF"), ("qtfp", 1, "SBUF"),
                ("qtbp", 2, "SBUF"), ("ktp", 2, "SBUF"), ("kop", 2, "SBUF"),
                ("knp", 1, "SBUF"), ("vnp", 2, "SBUF"), ("otp", 1, "SBUF"),
                ("tmpp", 2, "SBUF"), ("featp", 2, "SBUF"), ("zp", 2, "SBUF"),
                ("attnp", 3, "SBUF"), ("sp", 1, "SBUF"), ("ystp", 2, "SBUF"),
                ("ccp", 2, "SBUF"), ("dramp", 2, "DRAM"),
                ("ps_big", 2, "PSUM"), ("ps_r", 2, "PSUM"),
                ("ps_attn", 2, "PSUM"), ("ps_sm", 2, "PSUM"),
            ):
                pools[pname] = stack.enter_context(
                    tc.tile_pool(name=pname, bufs=pbufs, space=pspace))
            consts, xtp, qtfp, qtbp, ktp, kop, knp, vnp, otp, tmpp, featp, zp, \
                attnp, sp, ystp, ccp, dramp, ps_big, ps_r, ps_attn, ps_sm = (
                    pools[n] for n in (
                        "consts", "xtp", "qtfp", "qtbp", "ktp", "kop", "knp",
                        "vnp", "otp", "tmpp", "featp", "zp", "attnp", "sp",
                        "ystp", "ccp", "dramp", "ps_big", "ps_r", "ps_attn", "ps_sm"))
            # ---- constants / weights
            wq_s = consts.tile([128, KD, FL], bf16, tag="wq_s")
            wk_s = consts.tile([128, KD, D], bf16, tag="wk_s")
            wv_s = consts.tile([128, KD, FL], bf16, tag="wv_s")
            wo_s = consts.tile([128, 4, D], bf16, tag="wo_s")
            mask_s = consts.tile([128, 128], f32, tag="mask_s")
            mask2_s = consts.tile([128, 256], f32, tag="mask2_s")
            ident_s = consts.tile([128, 128], bf16, tag="ident_s")
            sel_other_s = consts.tile([128, 128], bf16, tag="sel_other_s")
            sel_sum_s = consts.tile([128, 128], bf16, tag="sel_sum_s")
            sel_f_s = consts.tile([128, 128], bf16, tag="sel_f_s")
            sel_lo_s = consts.tile([128, 128], bf16, tag="sel_lo_s")
            sel_hi_s = consts.tile([128, 128], bf16, tag="sel_hi_s")
            zsel_s = consts.tile([128, 128], bf16, tag="zsel_s")
            xtb0 = xtp.tile([128, KD, tb], bf16, tag="xtb", name="xtb0")
            nc.sync.dma_start(out=xtb0[:, :, 0:512], in_=xt[:, :, 0:512])
            nc.sync.dma_start(out=xtb0[:, :, 512:tb], in_=xt[:, :, 512:tb])
            for dst, src in ((wk_s, wk), (wq_s, wq), (wv_s, wv), (wo_s, wo),
                             (mask_s, mask_d), (ident_s, ident_d),
                             (mask2_s, mask2_d),
                             (sel_other_s, sel_other_d), (sel_sum_s, sel_sum_d),
                             (sel_f_s, sel_f_d), (sel_lo_s, sel_lo_d),
                             (sel_hi_s, sel_hi_d), (zsel_s, zsel_d)):
                nc.gpsimd.dma_start(out=dst, in_=src[tuple([slice(None)] * len(src.shape))])

            # ---- persistent scan state
            s_f32 = sp.tile([128, 4, DH], f32, tag="s_f32")
            s_bf = sp.tile([128, 4, DH], bf16, tag="s_bf")

            for rep in range(repeat):
              for bi in range(nblk):
                tsl = slice(bi * tb, (bi + 1) * tb)
                if bi == 0 and rep == 0:
                    xtb = xtb0
                else:
                    xtb = xtp.tile([128, KD, tb], bf16, tag="xtb")
                    nc.sync.dma_start(out=xtb, in_=xt[:, :, tsl])

                # ---- K-local pass: kt, R prefix, z, qtb
                kt = ktp.tile([128, 4, tb], bf16, tag="kt")
                qtb = qtbp.tile([128, 4, tb], bf16, tag="qtb")
                r_ps = [ps_r.tile([128, 512], f32, tag="ps_r", name=f"r_ps{j}")
                        for j in range(2)]
                if "kl" not in phases:
                    nc.vector.memset(kt, 0.01)
                nck = tb // 512
                for m in range(4 if "kl" in phases else 0):
                    for nch in range(nck):
                        csl = slice(nch * 512, (nch + 1) * 512)
                        pk = ps_big.tile([128, 512], f32, tag="ps_big")
                        for k in range(KD):
                            nc.tensor.matmul(pk, wk_s[:, k, (4 + m) * 128:(5 + m) * 128],
                                             xtb[:, k, csl], start=(k == 0), stop=(k == KD - 1))
                        _feature(nc, featp, pk, kt[:, m, csl], 512)
                if use_cc and "kl" in phases and "z" in phases:
                    s_own = ccp.tile([64, tb], bf16, tag="s_own")
                    for nch in range(nck):
                        csl = slice(nch * 512, (nch + 1) * 512)
                        sown = ps_big.tile([64, 512], f32, tag="ps_big", name="sown")
                        for m in range(4):
                            nc.tensor.matmul(sown, sel_sum_s[:, 0:64], kt[:, m, csl],
                                             start=(m == 0), stop=(m == 3))
                        nc.scalar.copy(s_own[:, csl], sown)
                    cc_in = dramp.tile([64, tb], bf16, tag="cc_in")
                    cc_out = dramp.tile([64, tb], bf16, tag="cc_out")
                    nc.sync.dma_start(out=cc_in[:, :], in_=s_own)
                    nc.gpsimd.collective_compute(
                        "AllReduce", ALU.add,
                        replica_groups=[[0, 1], [2, 3], [4, 5], [6, 7]],
                        ins=[cc_in[:, :]], outs=[cc_out[:, :]])
                    s_tot = ccp.tile([64, tb], bf16, tag="s_tot")
                    nc.sync.dma_start(out=s_tot, in_=cc_out[:, :])
                    s_oth = ccp.tile([64, tb], bf16, tag="s_oth")
                    nc.vector.tensor_sub(s_oth, s_tot, s_own)
                    for nch in range(nck):
                        csl = slice(nch * 512, (nch + 1) * 512)
                        nc.tensor.matmul(r_ps[nch], sel_f_s[0:64, :], s_oth[:, csl],
                                         start=True, stop=False, skip_group_check=True)
                # ---- Q projection + feature -> qt_f32 [128, 4, TB]
                qt_f32 = qtfp.tile([128, 4, tb], f32, tag="qt_f32")
                if "q" not in phases:
                    nc.vector.memset(qt_f32, 0.01)
                for m in range(4 if "q" in phases else 0):
                    for nch in range(tb // 512):
                        csl = slice(nch * 512, (nch + 1) * 512)
                        pq = ps_big.tile([128, 512], f32, tag="ps_big")
                        for k in range(KD):
                            nc.tensor.matmul(pq, wq_s[:, k, m * 128:(m + 1) * 128],
                                             xtb[:, k, csl], start=(k == 0), stop=(k == KD - 1))
                        _feature(nc, featp, pq, qt_f32[:, m, csl], 512)

                # ---- K-other pass -> R accumulation (R on both halves, in PSUM)
                if "z" not in phases:
                    nc.vector.tensor_copy(qtb, qt_f32)
                for m in range(4 if ("ko" in phases and not use_cc) else 0):
                    ko = kop.tile([128, tb], bf16, tag="ko")
                    for nch in range(tb // 512):
                        csl = slice(nch * 512, (nch + 1) * 512)
                        pk = ps_big.tile([128, 512], f32, tag="ps_big")
                        for k in range(KD):
                            nc.tensor.matmul(pk, wk_s[:, k, m * 128:(m + 1) * 128],
                                             xtb[:, k, csl], start=(k == 0), stop=(k == KD - 1))
                        _feature(nc, featp, pk, ko[:, csl], 512)
                    for nch in range(tb // 512 if "z" in phases else 0):
                        csl = slice(nch * 512, (nch + 1) * 512)
                        nc.tensor.matmul(r_ps[nch], sel_other_s, ko[:, csl],
                                         start=(m == 0), stop=False, skip_group_check=True)

                # ---- knat via PE transposes: [128(tok), NT, 512(feat)]
                knat = knp.tile([128, nt, FL], bf16, tag="knat")
                if "tr" not in phases:
                    nc.vector.memset(knat, 0.01)
                for tt in range(nt if "tr" in phases else 0):
                    ptr = ps_attn.tile([128, 512], bf16, tag="ps_attn")
                    for m in range(4):
                        nc.tensor.transpose(ptr[:, m * 128:(m + 1) * 128],
                                            kt[:, m, tt * 128:(tt + 1) * 128], ident_s)
                    nc.vector.tensor_copy(knat[:, tt, :], ptr)

                # ---- V pass -> vnat [128(tok), NT, 512]
                vnat = vnp.tile([128, nt, FL], bf16, tag="vnat")
                if "v" not in phases:
                    nc.vector.memset(vnat, 0.01)
                for tt in range(nt if "v" in phases else 0):
                    pv = ps_big.tile([128, 512], f32, tag="ps_big")
                    for k in range(KD):
                        nc.tensor.matmul(pv, xtb[:, k, tt * 128:(tt + 1) * 128],
                                         wv_s[:, k, :], start=(k == 0), stop=(k == KD - 1))
                    nc.scalar.copy(vnat[:, tt, :], pv)

                for m in range(4 if ("kl" in phases and "z" in phases) else 0):
                    if False:
                        pass
                    tmp = tmpp.tile([128, tb], bf16, tag="tmp")
                    for nch in range(tb // 512):
                        csl = slice(nch * 512, (nch + 1) * 512)
                        nc.tensor.matmul(r_ps[nch], sel_lo_s, kt[:, m, csl],
                                         start=(m == 0 and not use_cc and "ko" not in phases),
                                         stop=False, skip_group_check=True)
                        nc.vector.tensor_mul(tmp[0:64, csl], qt_f32[0:64, m, csl], r_ps[nch][0:64, :])
                    for nch in range(tb // 512):
                        csl = slice(nch * 512, (nch + 1) * 512)
                        nc.tensor.matmul(r_ps[nch], sel_hi_s, kt[:, m, csl],
                                         start=False, stop=(m == 3), skip_group_check=True)
                        nc.vector.tensor_mul(tmp[64:128, csl], qt_f32[64:128, m, csl], r_ps[nch][64:128, :])
                    for nch in range(tb // 512):
                        csl = slice(nch * 512, (nch + 1) * 512)
                        pz = ps_big.tile([128, 512], f32, tag="ps_big")
                        nc.tensor.matmul(pz, zsel_s, tmp[:, csl], start=True, stop=True)
                        zt = zp.tile([128, 512], f32, tag="zt")
                        nc.vector.reciprocal(zt, pz)
                        nc.gpsimd.tensor_mul(qtb[:, m, csl], qt_f32[:, m, csl], zt)

                # ---- causal scan
                outT = otp.tile([128, 4, tb], bf16, tag="outT")
                if "scan" not in phases:
                    nc.vector.memset(outT, 0.01)
                for tt in range(nt if "scan" in phases else 0):
                    c = bi * nt + tt
                    if c > 0:
                        nc.gpsimd.tensor_copy(s_bf, s_f32)
                    csl = slice(tt * 128, (tt + 1) * 128)
                    for pl in range(4):
                        pp = ps_sm.tile([128, 128], f32, tag="ps_sm")
                        attn_pair = []
                        for h in range(2):
                            i = 2 * pl + h
                            b64 = 64 * h
                            pa = ps_attn.tile([128, 128], f32, tag="ps_attn", name=f"pa{h}")
                            nc.tensor.matmul(pa, kt[b64:b64 + 64, pl, csl],
                                             qtb[b64:b64 + 64, pl, csl], start=True, stop=True)
                            attn_s = attnp.tile([128, 128], bf16, tag="attn_s", name=f"attn_s{h}")
                            nc.vector.tensor_mul(attn_s, pa, mask_s)
                            attn_pair.append(attn_s)
                        for h in range(2):
                            i = 2 * pl + h
                            b64 = 64 * h
                            nc.tensor.matmul(pp[b64:b64 + 64, :],
                                             vnat[:, tt, i * 64:(i + 1) * 64], attn_pair[h],
                                             start=True, stop=(c == 0), tile_position=(0, b64))
                            if c > 0:
                                nc.tensor.matmul(pp[b64:b64 + 64, :], s_bf[b64:b64 + 64, pl, :],
                                                 qtb[b64:b64 + 64, pl, csl],
                                                 start=False, stop=True, tile_position=(b64, b64))
                        nc.scalar.copy(outT[:, pl, csl], pp)
                        pd = ps_sm.tile([128, 64], f32, tag="ps_sm")
                        for h in range(2):
                            i = 2 * pl + h
                            b64 = 64 * h
                            nc.tensor.matmul(pd[b64:b64 + 64, :], knat[:, tt, i * 64:(i + 1) * 64],
                                             vnat[:, tt, i * 64:(i + 1) * 64],
                                             start=True, stop=True, tile_position=(0, b64))
                        if c == 0:
                            nc.vector.tensor_copy(s_f32[:, pl, :], pd)
                        else:
                            nc.vector.tensor_add(s_f32[:, pl, :], s_f32[:, pl, :], pd)

                # ---- output projection (partial over local features)
                for tt in range(nt if "proj" in phases else 0):
                    yst = ystp.tile([128, D], f32, tag="yst")
                    for nch in range(D // 512):
                        csl = slice(nch * 512, (nch + 1) * 512)
                        py = ps_big.tile([128, 512], f32, tag="ps_big")
                        for kf in range(4):
                            nc.tensor.matmul(py, outT[:, kf, tt * 128:(tt + 1) * 128],
                                             wo_s[:, kf, csl], start=(kf == 0), stop=(kf == 3))
                        nc.scalar.copy(yst[:, csl], py)
                    nc.sync.dma_start(out=y[bi * tb + tt * 128: bi * tb + (tt + 1) * 128, :],
                                      in_=yst)
    nc.finalize()
    return nc


_NC_CACHE = None


def _get_nc():
    global _NC_CACHE
    if _NC_CACHE is None:
        _NC_CACHE = build_nc()
    return _NC_CACHE


def _pack(w, kt):
    """[kt*128, N] -> [128, kt, N] contiguous."""
    return np.ascontiguousarray(w.reshape(kt, 128, -1).transpose(1, 0, 2))


def _core_inputs(x, Wq, Wk, Wv, Wo, core):
    b, s = core // 2, core % 2
    loc = np.arange(8 * s * DH, (8 * s + 8) * DH)
    oth = np.arange(8 * (1 - s) * DH, (8 * (1 - s) + 8) * DH)
    c16 = lambda a: np.ascontiguousarray(a.astype(ml_dtypes.bfloat16))

    I = np.eye(64, dtype=np.float32)
    Z = np.zeros((64, 64), dtype=np.float32)
    O = np.ones((64, 64), dtype=np.float32)
    flag = float(s)
    return {
        "xt": c16(_pack(x[b].T, KD)),
        "wq": c16(_pack(Wq[:, loc], KD)),
        "wk": c16(_pack(Wk[:, np.concatenate([oth, loc])], KD)),
        "wv": c16(_pack(Wv[:, loc], KD)),
        "wo": c16(_pack(Wo[loc, :], 4)),
        "mask": np.ascontiguousarray(np.triu(np.ones((128, 128), dtype=np.float32))),
        "mask2": np.ascontiguousarray(np.tile(np.triu(np.ones((128, 128), dtype=np.float32)), (1, 2))),
        "ident": c16(np.eye(128, dtype=np.float32)),
        "sel_other": c16(flag * np.block([[I, I], [I, I]])),
        "sel_sum": c16(np.block([[I, Z], [I, Z]])),
        "sel_f": c16(flag * np.block([[I, I], [Z, Z]])),
        "sel_lo": c16(np.block([[I, I], [Z, Z]])),
        "sel_hi": c16(np.block([[Z, Z], [I, I]])),
        "zsel": c16(np.block([[O, Z], [Z, O]])),
    }


def kernel(x, Wq, Wk, Wv, Wo, bo):
    x = np.asarray(x, dtype=np.float32)
    Wq = np.asarray(Wq, dtype=np.float32)
    Wk = np.asarray(Wk, dtype=np.float32)
    Wv = np.asarray(Wv, dtype=np.float32)
    Wo = np.asarray(Wo, dtype=np.float32)
    bo = np.asarray(bo, dtype=np.float32)

    nc = _get_nc()
    in_maps = [_core_inputs(x, Wq, Wk, Wv, Wo, core) for core in range(8)]
    res = run_bass_kernel_spmd(nc, in_maps, core_ids=list(range(8)))
    out = np.zeros((B, T, D), dtype=np.float32)
    for b in range(B):
        out[b] = (res.results[2 * b]["y"].astype(np.float32)
                  + res.results[2 * b + 1]["y"].astype(np.float32))
    out += bo
    return out
